# revision 2
# baseline (speedup 1.0000x reference)
"""GCN forward kernel for 8 Trainium2 NeuronCores (Bass/Tile).

    h   = BN1(leaky_relu(x @ W1 + b1))
    h2  = BN2(leaky_relu(gcn_conv(h) @ Wc + bc))
    out = log_softmax(concat(h, h2) @ W2 + b2)

Layout strategy (v2):
  - Nodes sharded over 8 cores; within each shard, nodes are PERMUTED by
    in-degree (host-side) so the edge aggregation can accumulate directly
    into SBUF with no scatter. The inverse permutation is applied to the
    output on the host.
  - x is fed pre-transposed in bf16 ([F_IN, NPAD] per core), so stage 1
    needs no on-device transposes.
  - hT lives in SBUF for the whole kernel (bf16, with an appended ones
    row so bias / constant terms fold into the matmuls as an extra
    weight row).
  - xl = BN1(h) @ Wc (+ bi1@Wc folded in) is exchanged in bf16 via
    AllGather; edges gather 4-node quads (256 B) from it with int16 quad
    indices in a single pass, weighted by a per-(edge,quadpos) norm table
    (zeros select the wanted node), then a per-dst-group reduction gives
    the conv output in SBUF.
  - log_softmax is computed with one Exp and one Ln activation-table
    load (no per-tile table thrash); BN2's allreduce latency is hidden
    behind the W2a matmuls and h2 transposes.

Self-contained: builds the Bass program from the actual inputs each call,
runs SPMD on cores 0-7, reassembles the full output on the host.
"""

import sys

sys.path.insert(0, "/opt/trn_rl_repo")

import numpy as np
import ml_dtypes
import concourse.bass as bass
import concourse.bacc as bacc
import concourse.mybir as mybir
import concourse.tile as tile
from concourse.bass_utils import run_bass_kernel_spmd
from concourse.masks import make_identity

NCORES = 8
EPS = 1e-5
SLOPE = 0.01
F32 = mybir.dt.float32
BF16 = mybir.dt.bfloat16
I16 = mybir.dt.int16
I8 = mybir.dt.int8
FP8 = mybir.dt.float8e4
AF = mybir.ActivationFunctionType
OP = mybir.AluOpType
NPBF = np.dtype(ml_dtypes.bfloat16)
NPF8 = np.dtype(ml_dtypes.float8_e4m3)
W1SCALE = 16.0

# ---------------------------------------------------------------------------
# this walrus build allows at most ONE sync-wait per instruction; spread
# extra waits over nops inserted before the instruction on the same engine.
_MAXW = 1


def _split_multi_waits(nc):
    for bb in nc.main_func.blocks:
        insts = bb.instructions
        i = 0
        while i < len(insts):
            inst = insts[i]
            si = inst.sync_info
            waits = list(si.on_wait) if si is not None else []
            if len(waits) > _MAXW:
                si.on_wait = waits[-_MAXW:]
                extra = waits[:-_MAXW]
                pos = i
                for j in range(0, len(extra), _MAXW):
                    nop = mybir.InstNoOp(
                        name=f"waitsplit-{nc.next_id()}",
                        sync_info=mybir.SyncInfo(
                            on_wait=extra[j : j + _MAXW], on_update=[]
                        ),
                        bass_nofuse=True,
                        engine=inst.engine,
                    )
                    insts.insert(pos, nop)
                    pos += 1
                    i += 1
            i += 1


def _finish(nc):
    nc.compile()
    _split_multi_waits(nc)
    bass.Bass.finalize(nc)


# ---------------------------------------------------------------------------
# host-side graph preprocessing


def _preprocess(N, edge_index, edge_weight):
    NSH = N // NCORES
    G = (NSH + 127) // 128
    NPAD = G * 128

    row = np.asarray(edge_index[0], dtype=np.int64)
    col = np.asarray(edge_index[1], dtype=np.int64)
    w = np.asarray(edge_weight, dtype=np.float64)

    deg = np.bincount(col, weights=w, minlength=N) + 1.0
    dinv = 1.0 / np.sqrt(deg)
    norm = (dinv[row] * w * dinv[col]).astype(np.float32)

    indeg = np.bincount(col, minlength=N) + 1  # count incl self-loop

    # per-core degree-descending permutation; pos[g] = permuted slot of node g
    perms = []
    pos = np.empty(N, dtype=np.int64)
    for c in range(NCORES):
        d = indeg[c * NSH:(c + 1) * NSH]
        p = np.argsort(-d, kind="stable")
        perms.append(p)
        ip = np.empty(NSH, dtype=np.int64)
        ip[p] = np.arange(NSH)
        pos[c * NSH:(c + 1) * NSH] = c * NPAD + ip

    loops = np.arange(N, dtype=np.int64)
    rows_all = np.concatenate([row, loops])
    cols_all = np.concatenate([col, loops])
    norms_all = np.concatenate([norm, (dinv * dinv).astype(np.float32)])

    src_pos = pos[rows_all]
    src_q = (src_pos // 4).astype(np.int16)
    src_qp = src_pos % 4
    dest_core = cols_all // NSH
    dst_pos = pos[cols_all] - dest_core * NPAD  # 0..NSH-1 in perm order

    def wrap16(flat):
        return flat.reshape(-1, 16).T.copy()

    # canonical group schedule: elementwise max over cores (the SPMD
    # instruction stream embeds the chunk layout, so it must be shared)
    sched = np.zeros(G, dtype=np.int64)
    for c in range(NCORES):
        d_by_pos = indeg[c * NSH:(c + 1) * NSH][perms[c]]
        sched = np.maximum(sched, d_by_pos[::128][:G])
    offs = np.zeros(G + 1, dtype=np.int64)
    offs[1:] = np.cumsum(sched)
    S = int(offs[-1])
    CH = int(max(32, sched.max()))
    assert CH <= 128, f"group degree {CH} too large for gather chunk"
    canon_chunks = []
    cur = None
    for g in range(G):
        d = int(sched[g])
        if d == 0:
            continue
        if cur is None or cur[1] + d > CH:
            cur = [int(offs[g]), 0, []]
            canon_chunks.append(cur)
        cur[2].append((g, cur[1], d))
        cur[1] += d

    arrs = {}
    for c in range(NCORES):
        m = dest_core == c
        dp = dst_pos[m]
        qs = src_q[m]
        qp = src_qp[m]
        nm = norms_all[m]

        order = np.argsort(dp, kind="stable")
        dp_s, qs_s, qp_s, nm_s = dp[order], qs[order], qp[order], nm[order]
        first = np.r_[True, dp_s[1:] != dp_s[:-1]]
        idx_first = np.flatnonzero(first)
        runlen = np.diff(np.r_[idx_first, len(dp_s)])
        rank = np.arange(len(dp_s)) - np.repeat(idx_first, runlen)
        grow = dp_s % 128
        ggrp = dp_s // 128
        scol = offs[ggrp] + rank
        assert (rank < sched[ggrp]).all()

        idx_flat = np.zeros(S * 128, dtype=np.int16)
        idx_flat[scol * 128 + grow] = qs_s
        n1 = np.zeros((128, S), dtype=np.float32)
        n1[grow, scol] = nm_s
        q8 = np.zeros((128, S), dtype=np.int8)
        q8[grow, scol] = qp_s

        arrs[c] = {
            "eidx": wrap16(idx_flat),
            "nrm1": n1.astype(NPBF),
            "qp8": q8,
        }

    return dict(N=N, NSH=NSH, G=G, NPAD=NPAD, S=S, CH=CH,
                canon_chunks=canon_chunks, arrs=arrs, perms=perms)


# ---------------------------------------------------------------------------
# device program (SPMD; identical instruction stream on every core)


def _build(meta, F_IN, H1, H2, NC_, prep_trigger=False):
    N, NSH, G, NPAD, S, CH = (meta[k] for k in ("N", "NSH", "G", "NPAD", "S", "CH"))
    KC = F_IN // 128
    H1A = H1 + 1          # hT with ones row
    H2A = H2 + 1          # h2T with ones row

    nc = bacc.Bacc("TRN2", target_bir_lowering=False, debug=False,
                   num_devices=NCORES, dynamic_dma_scratch_size=49152,
                   num_swdge_queues=4)

    xT_in = nc.declare_dram_parameter("xT", [F_IN, NPAD], BF16, isOutput=False)
    w1_in = nc.declare_dram_parameter("W1", [F_IN, H1], BF16, isOutput=False)
    b1_in = nc.declare_dram_parameter("b1", [H1, 1], F32, isOutput=False)
    g1_in = nc.declare_dram_parameter("g1", [H1, 1], F32, isOutput=False)
    be1_in = nc.declare_dram_parameter("be1", [H1, 1], F32, isOutput=False)
    wc_in = nc.declare_dram_parameter("Wc", [H1, H2], F32, isOutput=False)
    bc_in = nc.declare_dram_parameter("bc", [1, H2], F32, isOutput=False)
    g2_in = nc.declare_dram_parameter("g2c", [H2, 1], F32, isOutput=False)
    be2_in = nc.declare_dram_parameter("be2c", [H2, 1], F32, isOutput=False)
    w2_in = nc.declare_dram_parameter("W2", [H1 + H2, NC_], F32, isOutput=False)
    b2_in = nc.declare_dram_parameter("b2", [1, NC_], F32, isOutput=False)
    ei_in = nc.declare_dram_parameter("eidx", [16, S * 8], I16, isOutput=False)
    n1_in = nc.declare_dram_parameter("nrm1", [128, S], BF16, isOutput=False)
    q8_in = nc.declare_dram_parameter("qp8", [128, S], I8, isOutput=False)
    out_t = nc.declare_dram_parameter("out", [NPAD, NC_], F32, isOutput=True)

    xl_local = nc.dram_tensor("xl_local", [NPAD, H2], BF16)
    xl_full = nc.dram_tensor("xl_full", [NPAD * NCORES, H2], BF16,
                             addr_space="Shared")
    bn1_i = nc.dram_tensor("bn1_i", [H1, 2], F32)
    bn1_o = nc.dram_tensor("bn1_o", [H1, 2], F32, addr_space="Shared")
    bn2_i = nc.dram_tensor("bn2_i", [1, 2 * H2], F32)
    bn2_o = nc.dram_tensor("bn2_o", [1, 2 * H2], F32, addr_space="Shared")

    rg = [list(range(NCORES))]
    RLAST = NSH - (G - 1) * 128   # valid rows in last node tile

    # stage-1 node chunking: groups of 4 tiles (512 nodes)
    widths = []
    c0 = 0
    while c0 < NPAD:
        w = min(512, NPAD - c0)
        widths.append((c0, w))
        c0 += w

    with tile.TileContext(nc) as tc:
        with (
            tc.tile_pool(name="pers", bufs=1) as pers,
            tc.tile_pool(name="work", bufs=3) as work,
            tc.tile_pool(name="mpool", bufs=1) as mpool,
            tc.tile_pool(name="psc", bufs=2, space="PSUM") as psc,
            tc.tile_pool(name="psp", bufs=1, space="PSUM") as psp,
        ):
            ident = pers.tile([128, 128], F32, tag="ident")
            make_identity(nc, ident[:])
            eit = pers.tile([128, S * 8], I16, tag="eit")
            for kk in range(8):
                nc.sync.dma_start(out=eit[kk * 16:(kk + 1) * 16, :], in_=ei_in[:])
            ones_c = pers.tile([128, 1], F32, tag="ones_c")
            nc.gpsimd.memset(ones_c[:], 1.0)
            ones_r = pers.tile([1, 128], F32, tag="ones_r")
            nc.gpsimd.memset(ones_r[:], 1.0)

            # persistent SBUF state
            hT = pers.tile([H1A, NPAD], BF16, tag="hT")
            nc.gpsimd.memset(hT[H1:H1A, :], 1.0)
            h2f = pers.tile([128, G * H2], F32, tag="h2f")

            # ---------------- stage 1: h = leaky(x@W1 + b1) -> hT + stats
            w1_sb = pers.tile([128, KC * H1], BF16, tag="w1")
            nc.sync.dma_start(
                out=w1_sb[:].rearrange("p (k m) -> p k m", k=KC),
                in_=w1_in[:].rearrange("(k p) m -> p k m", p=128),
            )
            b1_sb = pers.tile([H1, 1], F32, tag="b1")
            nc.sync.dma_start(out=b1_sb[:], in_=b1_in[:])
            s1 = pers.tile([H1, 1], F32, tag="s1")
            nc.gpsimd.memset(s1[:], 0.0)
            sq1 = pers.tile([H1, 1], F32, tag="sq1")
            nc.gpsimd.memset(sq1[:], 0.0)

            for (c0, w) in widths:
                xk = work.tile([128, KC * 512], BF16, tag="xk")
                for k in range(KC):
                    nc.sync.dma_start(out=xk[:, k * 512:k * 512 + w],
                                      in_=xT_in[k * 128:(k + 1) * 128, c0:c0 + w])
                hp = psc.tile([H1, 512], F32, tag="mm")
                for k in range(KC):
                    nc.tensor.matmul(out=hp[:, :w],
                                     lhsT=w1_sb[:, k * H1:(k + 1) * H1],
                                     rhs=xk[:, k * 512:k * 512 + w],
                                     start=(k == 0), stop=(k == KC - 1))
                hsl = work.tile([H1, 512], F32, tag="hsl")
                nc.scalar.activation(out=hsl[:, :w], in_=hp[:, :w],
                                     func=AF.Identity, bias=b1_sb[:])
                nc.vector.scalar_tensor_tensor(
                    out=hT[:H1, c0:c0 + w], in0=hsl[:, :w], scalar=SLOPE,
                    in1=hsl[:, :w], op0=OP.mult, op1=OP.max)
                v = min(w, max(0, NSH - c0))
                if v > 0:
                    st = work.tile([H1, 1], F32, tag="st")
                    nc.vector.tensor_reduce(out=st[:], in_=hT[:H1, c0:c0 + v],
                                            axis=mybir.AxisListType.X, op=OP.add)
                    nc.vector.tensor_tensor(out=s1[:], in0=s1[:], in1=st[:],
                                            op=OP.add)
                    sqs = work.tile([H1, 512], F32, tag="sqs")
                    sqt = work.tile([H1, 1], F32, tag="sqt")
                    nc.scalar.activation(out=sqs[:, :v], in_=hT[:H1, c0:c0 + v],
                                         func=AF.Square, accum_out=sqt[:])
                    nc.vector.tensor_tensor(out=sq1[:], in0=sq1[:], in1=sqt[:],
                                            op=OP.add)

            # ---------------- BN1 stats allreduce -> sc1 / bi1
            st1 = pers.tile([H1, 2], F32, tag="st1")
            nc.vector.tensor_copy(out=st1[:, 0:1], in_=s1[:])
            nc.vector.tensor_copy(out=st1[:, 1:2], in_=sq1[:])
            nc.sync.dma_start(out=bn1_i[:], in_=st1[:])
            nc.gpsimd.collective_compute("AllReduce", OP.add, replica_groups=rg,
                                         ins=[bn1_i[:]], outs=[bn1_o[:]])
            sr1 = pers.tile([H1, 2], F32, tag="sr1")
            nc.sync.dma_start(out=sr1[:], in_=bn1_o[:])
            mean1 = pers.tile([H1, 1], F32, tag="mean1")
            nc.scalar.mul(mean1[:], sr1[:, 0:1], 1.0 / N)
            var1 = pers.tile([H1, 1], F32, tag="var1")
            nc.scalar.mul(var1[:], sr1[:, 1:2], 1.0 / N)
            tmp1 = pers.tile([H1, 1], F32, tag="tmp1")
            nc.vector.tensor_tensor(out=tmp1[:], in0=mean1[:], in1=mean1[:],
                                    op=OP.mult)
            nc.vector.tensor_tensor(out=var1[:], in0=var1[:], in1=tmp1[:],
                                    op=OP.subtract)
            nc.vector.tensor_scalar_add(var1[:], var1[:], EPS)
            sd1 = pers.tile([H1, 1], F32, tag="sd1")
            nc.scalar.activation(out=sd1[:], in_=var1[:], func=AF.Sqrt)
            inv1 = pers.tile([H1, 1], F32, tag="inv1")
            nc.vector.reciprocal(out=inv1[:], in_=sd1[:])
            g1_sb = pers.tile([H1, 1], F32, tag="g1s")
            nc.sync.dma_start(out=g1_sb[:], in_=g1_in[:])
            be1_sb = pers.tile([H1, 1], F32, tag="be1s")
            nc.sync.dma_start(out=be1_sb[:], in_=be1_in[:])
            sc1 = pers.tile([H1, 1], F32, tag="sc1")
            nc.vector.tensor_tensor(out=sc1[:], in0=inv1[:], in1=g1_sb[:],
                                    op=OP.mult)
            bi1 = pers.tile([H1, 1], F32, tag="bi1")
            nc.vector.tensor_tensor(out=bi1[:], in0=mean1[:], in1=sc1[:],
                                    op=OP.mult)
            nc.vector.tensor_tensor(out=bi1[:], in0=be1_sb[:], in1=bi1[:],
                                    op=OP.subtract)

            # wcp_aug = [diag(sc1) @ Wc ; q = bi1 @ Wc]
            wc_sb = pers.tile([H1, H2], F32, tag="wc")
            nc.sync.dma_start(out=wc_sb[:], in_=wc_in[:])
            wcp = pers.tile([H1A, H2], BF16, tag="wcp")
            nc.vector.tensor_scalar_mul(wcp[:H1, :], wc_sb[:], sc1[:])
            qp_ = psc.tile([1, H2], F32, tag="sm")
            nc.tensor.matmul(out=qp_[:], lhsT=bi1[:], rhs=wc_sb[:], start=True,
                             stop=True)
            nc.vector.tensor_copy(out=wcp[H1:H1A, :], in_=qp_[:])

            # ---------------- xl = [h;1] @ wcp_aug -> batched store, allgather
            xl_sb = pers.tile([128, G * H2], BF16, tag="xl")
            for t in range(G):
                xp = psc.tile([128, H2], F32, tag="mm")
                nc.tensor.matmul(out=xp[:], lhsT=hT[:, t * 128:(t + 1) * 128],
                                 rhs=wcp[:], start=True, stop=True)
                nc.scalar.copy(out=xl_sb[:, t * H2:(t + 1) * H2], in_=xp[:])
            nc.sync.dma_start(
                out=xl_local[:].rearrange("(g p) f -> p g f", p=128),
                in_=xl_sb[:].rearrange("p (g f) -> p g f", g=G))
            nc.gpsimd.collective_compute("AllGather", OP.bypass, replica_groups=rg,
                                         ins=[xl_local[:]], outs=[xl_full[:]])

            # ---------------- edge aggregation: single quad-gather pass
            NBLK = NPAD * NCORES // 4
            src_ap = bass.AP(tensor=xl_full, offset=0,
                             ap=[[4 * H2, NBLK], [1, 4 * H2]])
            chunks = meta["canon_chunks"]
            gsem = ([nc.alloc_semaphore(f"gsem{q}") for q in range(4)]
                    if prep_trigger else None)
            for ci, (col0, cols, groups) in enumerate(chunks):
                qn = ci % 4
                n1t = work.tile([128, CH], BF16, tag="n1t")
                nc.sync.dma_start(out=n1t[:, :cols],
                                  in_=n1_in[:, col0:col0 + cols])
                q8t = work.tile([128, CH], I8, tag="q8t")
                nc.sync.dma_start(out=q8t[:, :cols],
                                  in_=q8_in[:, col0:col0 + cols])
                nt = work.tile([128, CH * 4], BF16, tag="nt")
                eqj = work.tile([128, CH], BF16, tag="eqj")
                for j in range(4):
                    nc.vector.tensor_scalar(out=eqj[:, :cols], in0=q8t[:, :cols],
                                            scalar1=float(j), scalar2=None,
                                            op0=OP.is_equal)
                    ntv = bass.AP(tensor=nt[:].tensor, offset=nt[:].offset + j,
                                  ap=[list(nt[:].ap[0]), [4, cols]])
                    nc.vector.tensor_tensor(out=ntv, in0=n1t[:, :cols],
                                            in1=eqj[:, :cols], op=OP.mult)
                ms = mpool.tile([128, CH * 4 * H2], BF16, tag=f"ms{qn}")
                gkw = (dict(prepare_only=True, sem=gsem[qn]) if prep_trigger
                       else {})
                nc.gpsimd.dma_gather(
                    out_ap=ms[:, :cols * 4 * H2].rearrange(
                        "p (m e) -> p m e", e=4 * H2),
                    in_ap=src_ap,
                    idxs_ap=eit[:, col0 * 8:(col0 + cols) * 8],
                    num_idxs=cols * 128, num_idxs_reg=cols * 128,
                    elem_size=4 * H2,
                    single_packet=False, queue_num=qn,
                    **gkw,
                )
                if prep_trigger:
                    nc.gpsimd.trigger_dma(count=None, queue_num=qn)
                mv = ms[:, :cols * 4 * H2].rearrange(
                    "p (m h f) -> p m h f", h=4, f=H2)
                nv = bass.AP(
                    tensor=nt[:].tensor,
                    offset=nt[:].offset,
                    ap=[list(nt[:].ap[0]), [4, cols], [1, 4], [0, H2]],
                )
                nc.vector.tensor_tensor(out=mv, in0=mv, in1=nv, op=OP.mult)
                # batch consecutive groups with equal degree into one reduce
                runs = []
                for (g, rel, d) in groups:
                    if (runs and runs[-1][2] == d and g == runs[-1][0] + runs[-1][3]
                            and rel == runs[-1][1] + runs[-1][3] * d):
                        runs[-1][3] += 1
                    else:
                        runs.append([g, rel, d, 1])
                for (g, rel, d, k) in runs:
                    rv = bass.AP(
                        tensor=ms[:].tensor,
                        offset=ms[:].offset + rel * 4 * H2,
                        ap=[list(ms[:].ap[0]), [4 * H2 * d, k], [1, H2],
                            [H2, 4 * d]],
                    )
                    nc.vector.tensor_reduce(
                        out=h2f[:, g * H2:(g + k) * H2], in_=rv,
                        axis=mybir.AxisListType.X, op=OP.add)

            # ---------------- h2 = leaky(h2f + bc); BN2 stats
            bc_sb = pers.tile([1, H2], F32, tag="bcr")
            nc.sync.dma_start(out=bc_sb[:], in_=bc_in[:])
            bcp = psc.tile([128, H2], F32, tag="sm")
            nc.tensor.matmul(out=bcp[:], lhsT=ones_r[:], rhs=bc_sb[:], start=True,
                             stop=True)
            bcb = pers.tile([128, H2], F32, tag="bcb")
            nc.vector.tensor_copy(out=bcb[:], in_=bcp[:])

            bcv = bass.AP(tensor=bcb[:].tensor, offset=bcb[:].offset,
                          ap=[list(bcb[:].ap[0]), [0, G], [1, H2]])
            h2v = h2f[:].rearrange("p (g f) -> p g f", g=G)
            nc.vector.tensor_tensor(out=h2v, in0=h2v, in1=bcv, op=OP.add)
            nc.vector.scalar_tensor_tensor(out=h2f[:], in0=h2f[:], scalar=SLOPE,
                                           in1=h2f[:], op0=OP.mult, op1=OP.max)
            if RLAST < 128:
                # zero pad rows of the last group (partition sub-ranges must
                # start at multiples of 32, so memset is out -> mask multiply)
                rmask = pers.tile([128, 1], F32, tag="rmask")
                nc.vector.tensor_reduce(out=rmask[:], in_=ident[:, :RLAST],
                                        axis=mybir.AxisListType.X, op=OP.add)
                nc.vector.tensor_scalar_mul(h2f[:, (G - 1) * H2:],
                                            h2f[:, (G - 1) * H2:], rmask[:])

            sqb = pers.tile([128, G * H2], BF16, tag="sqb")
            nc.scalar.activation(out=sqb[:], in_=h2f[:], func=AF.Square)
            ones_cb = pers.tile([128, 1], BF16, tag="ones_cb")
            nc.gpsimd.memset(ones_cb[:], 1.0)
            st2 = pers.tile([1, 2 * H2], F32, tag="st2")
            nc.gpsimd.memset(st2[:], 0.0)
            for (src, o) in ((h2f, 0), (sqb, H2)):
                cc0 = 0
                while cc0 < G * H2:
                    w = min(512, G * H2 - cc0)
                    sp = psp.tile([1, 512], F32, tag="sp")
                    lo = ones_c[:] if o == 0 else ones_cb[:]
                    nc.tensor.matmul(out=sp[:, :w], lhsT=lo,
                                     rhs=src[:, cc0:cc0 + w], start=True,
                                     stop=True)
                    pv = bass.AP(tensor=sp[:].tensor, offset=sp[:].offset,
                                 ap=[list(sp[:].ap[0]), [1, H2], [H2, w // H2]])
                    wt = work.tile([1, H2], F32, tag="wt")
                    nc.vector.tensor_reduce(out=wt[:], in_=pv,
                                            axis=mybir.AxisListType.X, op=OP.add)
                    nc.vector.tensor_tensor(out=st2[:, o:o + H2],
                                            in0=st2[:, o:o + H2], in1=wt[:],
                                            op=OP.add)
                    cc0 += w
            nc.sync.dma_start(out=bn2_i[:], in_=st2[:])
            nc.gpsimd.collective_compute("AllReduce", OP.add, replica_groups=rg,
                                         ins=[bn2_i[:]], outs=[bn2_o[:]])

            # ---------------- F1 (overlaps BN2 allreduce): W2a logits + h2T
            w2a_sb = pers.tile([H1, NC_], F32, tag="w2a")
            nc.sync.dma_start(out=w2a_sb[:], in_=w2_in[:H1, :])
            w2b_sb = pers.tile([H2, NC_], F32, tag="w2b")
            nc.sync.dma_start(out=w2b_sb[:], in_=w2_in[H1:, :])
            b2_sb = pers.tile([1, NC_], F32, tag="b2")
            nc.sync.dma_start(out=b2_sb[:], in_=b2_in[:])

            w2ap = pers.tile([H1A, NC_], BF16, tag="w2ap")
            nc.vector.tensor_scalar_mul(w2ap[:H1, :], w2a_sb[:], sc1[:])
            c0p = psc.tile([1, NC_], F32, tag="sm")
            nc.tensor.matmul(out=c0p[:], lhsT=bi1[:], rhs=w2a_sb[:], start=True,
                             stop=True)
            c0a = pers.tile([1, NC_], F32, tag="c0a")
            nc.vector.tensor_tensor(out=c0a[:], in0=c0p[:], in1=b2_sb[:],
                                    op=OP.add)
            nc.vector.tensor_copy(out=w2ap[H1:H1A, :], in_=c0a[:])

            h2T = pers.tile([H2A, G * 128], BF16, tag="h2T")
            nc.gpsimd.memset(h2T[H2:H2A, :], 1.0)
            lgA = pers.tile([128, G * NC_], F32, tag="lgA")
            for t in range(G):
                lg = psc.tile([128, NC_], F32, tag="mm")
                nc.tensor.matmul(out=lg[:], lhsT=hT[:, t * 128:(t + 1) * 128],
                                 rhs=w2ap[:], start=True, stop=True)
                nc.scalar.copy(out=lgA[:, t * NC_:(t + 1) * NC_], in_=lg[:])
                tp = psc.tile([H2, 128], F32, tag="tp")
                nc.tensor.transpose(out=tp[:], in_=h2f[:, t * H2:(t + 1) * H2],
                                    identity=ident[:])
                nc.vector.tensor_copy(out=h2T[:H2, t * 128:(t + 1) * 128],
                                      in_=tp[:])

            # ---------------- BN2 fold -> w2bp_aug
            sr2 = pers.tile([1, 2 * H2], F32, tag="sr2")
            nc.sync.dma_start(out=sr2[:], in_=bn2_o[:])
            sr2tp = psc.tile([2 * H2, 1], F32, tag="tp")
            nc.tensor.transpose(out=sr2tp[:], in_=sr2[:], identity=ident[:1, :1])
            sr2t = pers.tile([2 * H2, 1], F32, tag="sr2t")
            nc.vector.tensor_copy(out=sr2t[:], in_=sr2tp[:])
            mean2 = pers.tile([H2, 1], F32, tag="mean2")
            nc.scalar.mul(mean2[:], sr2t[:H2, :], 1.0 / N)
            var2 = pers.tile([H2, 1], F32, tag="var2")
            nc.scalar.mul(var2[:], sr2t[H2:, :], 1.0 / N)
            tmp2 = pers.tile([H2, 1], F32, tag="tmp2")
            nc.vector.tensor_tensor(out=tmp2[:], in0=mean2[:], in1=mean2[:],
                                    op=OP.mult)
            nc.vector.tensor_tensor(out=var2[:], in0=var2[:], in1=tmp2[:],
                                    op=OP.subtract)
            nc.vector.tensor_scalar_add(var2[:], var2[:], EPS)
            sd2 = pers.tile([H2, 1], F32, tag="sd2")
            nc.scalar.activation(out=sd2[:], in_=var2[:], func=AF.Sqrt)
            inv2 = pers.tile([H2, 1], F32, tag="inv2")
            nc.vector.reciprocal(out=inv2[:], in_=sd2[:])
            g2_sb = pers.tile([H2, 1], F32, tag="g2s")
            nc.sync.dma_start(out=g2_sb[:], in_=g2_in[:])
            be2_sb = pers.tile([H2, 1], F32, tag="be2s")
            nc.sync.dma_start(out=be2_sb[:], in_=be2_in[:])
            sc2 = pers.tile([H2, 1], F32, tag="sc2")
            nc.vector.tensor_tensor(out=sc2[:], in0=inv2[:], in1=g2_sb[:],
                                    op=OP.mult)
            bi2 = pers.tile([H2, 1], F32, tag="bi2")
            nc.vector.tensor_tensor(out=bi2[:], in0=mean2[:], in1=sc2[:],
                                    op=OP.mult)
            nc.vector.tensor_tensor(out=bi2[:], in0=be2_sb[:], in1=bi2[:],
                                    op=OP.subtract)

            w2bp = pers.tile([H2A, NC_], BF16, tag="w2bp")
            nc.vector.tensor_scalar_mul(w2bp[:H2, :], w2b_sb[:], sc2[:])
            c1p = psc.tile([1, NC_], F32, tag="sm")
            nc.tensor.matmul(out=c1p[:], lhsT=bi2[:], rhs=w2b_sb[:], start=True,
                             stop=True)
            nc.vector.tensor_copy(out=w2bp[H2:H2A, :], in_=c1p[:])

            # ---------------- F2: logits, Exp accumulate; F3: - Ln(sum)
            xm_all = pers.tile([128, G * NC_], F32, tag="xm")
            ses = pers.tile([128, G], F32, tag="ses")
            for t in range(G):
                lg2 = psc.tile([128, NC_], F32, tag="mm")
                nc.tensor.matmul(out=lg2[:], lhsT=h2T[:, t * 128:(t + 1) * 128],
                                 rhs=w2bp[:], start=True, stop=True)
                sl = xm_all[:, t * NC_:(t + 1) * NC_]
                nc.vector.tensor_tensor(out=sl, in0=lg2[:],
                                        in1=lgA[:, t * NC_:(t + 1) * NC_],
                                        op=OP.add)
                ex = work.tile([128, NC_], F32, tag="ex")
                nc.scalar.activation(out=ex[:], in_=sl, func=AF.Exp,
                                     accum_out=ses[:, t:t + 1])
            ls_all = pers.tile([128, G], F32, tag="ls")
            nc.scalar.activation(out=ls_all[:], in_=ses[:], func=AF.Ln)
            for t in range(G):
                sl = xm_all[:, t * NC_:(t + 1) * NC_]
                nc.vector.tensor_scalar_sub(sl, sl, ls_all[:, t:t + 1])
            nc.sync.dma_start(
                out=out_t[:].rearrange("(g p) f -> p g f", p=128),
                in_=xm_all[:].rearrange("p (g f) -> p g f", g=G))

    _finish(nc)
    return nc


# ---------------------------------------------------------------------------


def make_in_maps(meta, x, W1, b1, g1, be1, Wc, bc, g2, be2, W2, b2):
    x = np.asarray(x, dtype=np.float32)
    F_IN = x.shape[1]
    NSH, NPAD = meta["NSH"], meta["NPAD"]
    in_maps = []
    for c in range(NCORES):
        shard = x[c * NSH:(c + 1) * NSH][meta["perms"][c]]
        xt = np.zeros((F_IN, NPAD), dtype=NPBF)
        xt[:, :NSH] = shard.T.astype(NPBF)
        m = {
            "xT": xt,
            "W1": np.asarray(W1, np.float32).astype(NPBF),
            "b1": np.asarray(b1, np.float32).reshape(-1, 1),
            "g1": np.asarray(g1, np.float32).reshape(-1, 1),
            "be1": np.asarray(be1, np.float32).reshape(-1, 1),
            "Wc": np.asarray(Wc, np.float32),
            "bc": np.asarray(bc, np.float32).reshape(1, -1),
            "g2c": np.asarray(g2, np.float32).reshape(-1, 1),
            "be2c": np.asarray(be2, np.float32).reshape(-1, 1),
            "W2": np.asarray(W2, np.float32),
            "b2": np.asarray(b2, np.float32).reshape(1, -1),
        }
        m.update(meta["arrs"][c])
        in_maps.append(m)
    return in_maps


def kernel(x, edge_index, edge_weight, W1, b1, g1, be1, Wc, bc, g2, be2, W2, b2):
    x = np.asarray(x, dtype=np.float32)
    N, F_IN = x.shape
    H1 = np.asarray(W1).shape[1]
    H2 = np.asarray(Wc).shape[1]
    NC_ = np.asarray(W2).shape[1]

    meta = _preprocess(N, edge_index, edge_weight)
    NSH, NPAD = meta["NSH"], meta["NPAD"]

    nc = _build(meta, F_IN, H1, H2, NC_)
    in_maps = make_in_maps(meta, x, W1, b1, g1, be1, Wc, bc, g2, be2, W2, b2)

    res = run_bass_kernel_spmd(nc, in_maps, list(range(NCORES)))
    outs = []
    for c in range(NCORES):
        dev = res.results[c]["out"][:NSH]
        nat = np.empty_like(dev)
        nat[meta["perms"][c]] = dev
        outs.append(nat)
    return np.concatenate(outs, axis=0)

